# revision 12
# baseline (speedup 1.0000x reference)
"""ContextualAttention TRN2 kernel (8 NeuronCores, SPMD).

Sharding: core = (sample s, query-half h); s = core // 2, h = core % 2.
Each core handles its sample's scores/softmax/attention for queries in rows
[32h, 32h+32) of the 64x64 query grid (plus a +-64-query halo for the
conv-transpose fold) and the argmax ("offsets") for its own rows.

Device pipeline (all heavy matmuls f32r = 1 cyc/row on the PE):
  P0  fm = f_slice * (1 - mp_s) (broadcast via GPSIMD), b rounded + PE-
      transposed, staged to DRAM as b^T padded.
  P1  patch bank: per l-tile, gather 9 shifted b^T slices, * m0 (stride-0
      broadcast) -> w^T [l, (d,c)] + ones col; norm^2 via ACT Square-accum;
      rn = 1/sqrt(n2+eps); wn^T = w^T * rn; PE-transpose -> wn [(d,c), l];
      w^T tile streamed to DRAM.  fp_unf built by 9 shifted DMAs from fm.
  P2  GEMM1a: score[q, l] per q-tile -> DVE top-8 max / argmax; global max
      -> softmax shift bias = 40 - 10*gmax.
  P3  per q-chunk x l-tile: GEMM1b score^T[l, q] -> ACT exp(10*s + bias) ->
      e^T (f32r); GEMM2 accumulates G[q, (d,c)|den] = e^T.T @ w^T; then
      H = G * (1/den).
  P4  fold: y^T[p, c] = sum_d H[p - dlin(d), (d,c)] via masked shifted-
      diagonal PE matmuls (host-built masks encode x-wrap + sample edges).

Host: shards inputs, pools the masks (65k flops), builds constants,
reassembles outputs, and re-resolves argmax for near-tie queries
(device top-2 gap < 3e-3) with an exact fp64 rescore.
"""
import numpy as np
from contextlib import ExitStack

import concourse.bass as bass
import concourse.bacc as bacc
import concourse.tile as tile
import concourse.mybir as mybir
import concourse.bass_isa as bass_isa
from concourse.bass_utils import run_bass_kernel_spmd

F32 = mybir.dt.float32
F32R = mybir.dt.float32r
U32 = mybir.dt.uint32
AF = mybir.ActivationFunctionType
ALU = mybir.AluOpType

B, C, H, W = 4, 64, 64, 64
L = H * W                       # 4096
NCORES = 8
KD = 9 * C                      # 576
KCH = [128, 128, 128, 128, 64]
NT = 32                         # l-tiles
QOWN = L // 2                   # 2048 own queries per core
QH = QOWN + 2 * W               # 2176 incl halo
NQT = QH // 128                 # 17
FSL = 2432                      # f slice width: q in [q0-192, q0+2240)
CHUNKS = [384, 384, 384, 384, 384, 256]
SCALE = 10.0
EPS = 1e-4
DELTAS = [(di, dj) for di in (-1, 0, 1) for dj in (-1, 0, 1)]
GAP_RESCUE = 3e-3

# fold piece table: per delta, shift sh = 64 - dlin and the two source-tile
# offsets (relative to the output y-tile index in H-local tiles)
def _fold_offs(sh):
    if sh == -1:
        return (-1, 0)
    if 0 <= sh <= 127:
        return (0, 1)
    return (1, 2)  # sh in {128, 129}

_BUILT = None
_LAST_RES = None


def _build():
    nc = bacc.Bacc("TRN2", target_bir_lowering=False, debug=False,
                   num_devices=NCORES)

    d_b = nc.dram_tensor("b_in", [C, L], F32, kind="ExternalInput").ap()
    d_f = nc.dram_tensor("f_sl", [C, FSL], F32, kind="ExternalInput").ap()
    d_om = nc.dram_tensor("onem", [1, FSL], F32, kind="ExternalInput").ap()
    d_mp0 = nc.dram_tensor("mp0pad", [66 * 66], F32, kind="ExternalInput").ap()
    d_id = nc.dram_tensor("identr", [128, 128], F32R, kind="ExternalInput").ap()
    d_1s = nc.dram_tensor("onesr", [128, 32], F32R, kind="ExternalInput").ap()
    d_fmx = nc.dram_tensor("foldm", [54, 128, 128], F32R, kind="ExternalInput").ap()

    d_y = nc.dram_tensor("y_out", [128, 16 * 64], F32, kind="ExternalOutput").ap()
    d_ix = nc.dram_tensor("idx_out", [128, 16 * 8], U32, kind="ExternalOutput").ap()
    d_mx = nc.dram_tensor("max_out", [128, 16 * 8], F32, kind="ExternalOutput").ap()

    d_bpT = nc.dram_tensor("bpadT", [(L + 4 * W) * C], F32R, kind="Internal").ap()
    d_wT = nc.dram_tensor("wT_dram", [NT, 128, KD + 4], F32R, kind="Internal").ap()

    with tile.TileContext(nc) as tc, ExitStack() as ctx:
        cst = ctx.enter_context(tc.tile_pool(name="cst", bufs=1))
        t_id = cst.tile([128, 128], F32R, tag="ident")
        nc.sync.dma_start(t_id[:], d_id)
        t_1s = cst.tile([128, 32], F32R, tag="onesr")
        nc.sync.dma_start(t_1s[:], d_1s)
        t_zer = cst.tile([128, KD], F32R, tag="zerot")
        nc.vector.memset(t_zer[:].bitcast(U32), 0)

        bigp = ctx.enter_context(tc.tile_pool(name="bigp", bufs=1))
        t_wn = bigp.tile([128, 5 * L], F32R, tag="wn")
        t_fpu = bigp.tile([128, 5 * QH], F32R, tag="fpu")

        # ---------------- P0: fm + b staging ----------------
        with tc.tile_pool(name="pre", bufs=1) as pre, \
             tc.tile_pool(name="pps", bufs=3, space="PSUM") as pps:
            t_fmp = pre.tile([64, FSL], F32R, tag="fmp")
            t_f = pre.tile([64, FSL], F32, tag="fload")
            nc.sync.dma_start(t_f[:], d_f)
            t_omr = pre.tile([1, FSL], F32, tag="omrow")
            nc.sync.dma_start(t_omr[:], d_om)
            t_omb = pre.tile([64, FSL], F32, tag="omrep")
            nc.gpsimd.partition_broadcast(t_omb[:], t_omr[:], channels=64)
            nc.vector.tensor_tensor(t_fmp[:], t_f[:], t_omb[:], ALU.mult)

            t_b = pre.tile([64, L], F32, tag="bload")
            nc.sync.dma_start(t_b[:], d_b)
            t_br = pre.tile([64, L], F32R, tag="bround")
            nc.vector.tensor_copy(t_br[:], t_b[:])
            t_bT = pre.tile([128, NT * 64], F32R, tag="bT")
            for t in range(NT):
                pt = pps.tile([128, 64], F32R, tag="ptr0")
                nc.tensor.transpose(pt[:], t_br[:, t * 128:(t + 1) * 128], t_id[0:64, 0:64])
                if t % 2 == 0:
                    nc.vector.tensor_copy(t_bT[:, t * 64:(t + 1) * 64], pt[:])
                else:
                    nc.scalar.copy(t_bT[:, t * 64:(t + 1) * 64], pt[:])
            # bpadT[64 + l, c] with zero pad rows
            nc.sync.dma_start(
                bass.AP(tensor=d_bpT.tensor, offset=0, ap=[[64, 128], [1, 64]]),
                t_zer[:, 0:64])
            nc.sync.dma_start(
                bass.AP(tensor=d_bpT.tensor, offset=(2 * W + L) * C,
                        ap=[[64, 128], [1, 64]]),
                t_zer[:, 0:64])
            nc.sync.dma_start(
                bass.AP(tensor=d_bpT.tensor, offset=2 * W * C,
                        ap=[[C, 128], [128 * C, NT], [1, 64]]),
                t_bT[:].rearrange("p (t c) -> p t c", t=NT))

            # fp_unf: 9 shifted DMAs from fm (+ x-wrap zeroing)
            for d, (di, dj) in enumerate(DELTAS):
                dlin = di * W + dj
                kc, half = divmod(d, 2)
                r0 = half * 64
                nc.sync.dma_start(
                    t_fpu[r0:r0 + 64, kc * QH:kc * QH + QH],
                    t_fmp[:, 128 + dlin:128 + dlin + QH])
            for d, (di, dj) in enumerate(DELTAS):
                if dj == 0:
                    continue
                kc, half = divmod(d, 2)
                r0 = half * 64
                col0 = 63 if dj == 1 else 0
                nc.vector.memset(
                    t_fpu[r0:r0 + 64, kc * QH + col0:kc * QH + QH:64].bitcast(U32), 0)

        # ---------------- P1: patch bank ----------------
        with tc.tile_pool(name="p1", bufs=1) as p1, \
             tc.tile_pool(name="btg", bufs=3) as btg, \
             tc.tile_pool(name="wtt", bufs=3) as wtt, \
             tc.tile_pool(name="wps", bufs=4, space="PSUM") as wps:
            t_m0T = p1.tile([128, 9 * NT], F32, tag="m0T")
            for d, (di, dj) in enumerate(DELTAS):
                off0 = (1 + di) * 66 + (1 + dj)
                for ph in range(2):
                    g = bass.AP(tensor=d_mp0.tensor, offset=off0 + ph * 66,
                                ap=[[1, 64], [132, 32]])
                    nc.sync.dma_start(
                        t_m0T[ph * 64:(ph + 1) * 64, d * 32:(d + 1) * 32], g)

            t_n2 = p1.tile([128, NT], F32, tag="n2")
            t_rn = p1.tile([128, NT], F32, tag="rn")
            t_eps = p1.tile([128, 1], F32, tag="eps")
            nc.vector.memset(t_eps[:], EPS)
            for t in range(NT):
                bt = btg.tile([128, KD], F32R, tag="btg")
                for d, (di, dj) in enumerate(DELTAS):
                    dlin = di * W + dj
                    nc.sync.dma_start(
                        bt[:, d * 64:(d + 1) * 64],
                        bass.AP(tensor=d_bpT.tensor,
                                offset=(2 * W + dlin + 128 * t) * C,
                                ap=[[C, 128], [1, 64]]))
                wt = wtt.tile([128, KD + 4], F32R, tag="wTt")
                nc.vector.memset(wt[:, KD + 1:KD + 4].bitcast(U32), 0)
                nc.vector.tensor_tensor(
                    wt[:, 0:KD].rearrange("p (d c) -> p d c", d=9),
                    bt[:].rearrange("p (d c) -> p d c", d=9),
                    t_m0T[:, t:9 * NT:NT].rearrange("p (d u) -> p d u", u=1)
                        .broadcast_to([128, 9, 64]),
                    ALU.mult)
                nc.sync.dma_start(wt[:, KD:KD + 1], t_1s[:, 0:1])
                sq = wtt.tile([128, KD], F32, tag="sqscr")
                nc.scalar.activation(sq[:], wt[:, 0:KD], AF.Square,
                                     bias=0.0, scale=1.0,
                                     accum_out=t_n2[:, t:t + 1])
                nc.scalar.activation(t_rn[:, t:t + 1], t_n2[:, t:t + 1], AF.Sqrt,
                                     bias=t_eps[:], scale=1.0)
                nc.vector.reciprocal(t_rn[:, t:t + 1], t_rn[:, t:t + 1])
                wnt = wtt.tile([128, KD], F32R, tag="wnTt")
                nc.vector.tensor_scalar_mul(wnt[:], wt[:, 0:KD], t_rn[:, t:t + 1])
                for kc in range(5):
                    kb = KCH[kc]
                    ptr = wps.tile([128, 128], F32R, tag="ptrw")
                    nc.tensor.transpose(ptr[0:kb, :],
                                        wnt[:, kc * 128:kc * 128 + kb], t_id[:])
                    dst = t_wn[0:kb, kc * L + t * 128:kc * L + (t + 1) * 128]
                    if t % 2 == 0:
                        nc.vector.tensor_copy(dst, ptr[0:kb, :])
                    else:
                        nc.scalar.copy(dst, ptr[0:kb, :])
                nc.sync.dma_start(d_wT[t], wt[:])

        # ---------------- P2: score[q, l] + argmax ----------------
        t_bias = cst.tile([128, 1], F32, tag="bias")
        with tc.tile_pool(name="p2", bufs=2) as p2, \
             tc.tile_pool(name="p2s", bufs=1) as p2s, \
             tc.tile_pool(name="ps2", bufs=8, space="PSUM") as ps2:
            t_mxa = p2s.tile([128, 16 * 8], F32, tag="mxall")
            t_ixa = p2s.tile([128, 16 * 8], U32, tag="ixall")
            for qt in range(16):
                sc = p2.tile([128, L], F32, tag="scq")
                for lc in range(8):
                    ps = ps2.tile([128, 512], F32, tag="scps")
                    for kc in range(5):
                        kb = KCH[kc]
                        nc.tensor.matmul(
                            ps[:],
                            t_fpu[0:kb, kc * QH + 64 + qt * 128:
                                  kc * QH + 64 + (qt + 1) * 128],
                            t_wn[0:kb, kc * L + lc * 512:kc * L + (lc + 1) * 512],
                            start=(kc == 0), stop=(kc == 4))
                    nc.scalar.copy(sc[:, lc * 512:(lc + 1) * 512], ps[:])
                nc.vector.max(t_mxa[:, qt * 8:(qt + 1) * 8], sc[:])
                nc.vector.max_index(t_ixa[:, qt * 8:(qt + 1) * 8],
                                    t_mxa[:, qt * 8:(qt + 1) * 8], sc[:])
            nc.sync.dma_start(d_ix, t_ixa[:])
            nc.sync.dma_start(d_mx, t_mxa[:])
            # global max -> exp bias = 40 - 10*gmax
            t_g8 = p2s.tile([128, 8], F32, tag="g8")
            nc.vector.max(t_g8[:], t_mxa[:])
            t_gr = p2s.tile([128, 1], F32, tag="gmaxr")
            nc.gpsimd.partition_all_reduce(t_gr[:], t_g8[:, 0:1], channels=128,
                                           reduce_op=bass_isa.ReduceOp.max)
            nc.vector.tensor_scalar(t_bias[:], t_gr[:], -SCALE, 40.0,
                                    ALU.mult, ALU.add)

        # ---------------- P3: exp + GEMM2 + H; P4: fold ----------------
        fold_state = {"n": 0}
        h_tiles = []
        t_ysb = cst.tile([128, 16 * 64], F32, tag="ysb")

        with tc.tile_pool(name="fmx", bufs=1) as fmxp, \
             tc.tile_pool(name="ep", bufs=3) as ep, \
             tc.tile_pool(name="ws", bufs=3) as ws, \
             tc.tile_pool(name="hp", bufs=6) as hp, \
             tc.tile_pool(name="rp", bufs=4) as rp, \
             tc.tile_pool(name="ps3", bufs=2, space="PSUM") as ps3, \
             tc.tile_pool(name="psA", bufs=3, space="PSUM") as psA, \
             tc.tile_pool(name="psB", bufs=3, space="PSUM") as psB:
            t_fmx = fmxp.tile([128, 54 * 128], F32R, tag="foldm")
            nc.sync.dma_start(
                t_fmx[:].rearrange("p (m k) -> p m k", m=54),
                d_fmx.rearrange("m p k -> p m k"))

            def fold(t):
                # shares the "s1ps" slots with GEMM1b psum (8-bank budget)
                py = ps3.tile([128, 64], F32, tag="s1ps")
                first = True
                set_idx = 0 if t == 0 else (2 if t == 15 else 1)
                cnt = 0
                for d, (di, dj) in enumerate(DELTAS):
                    sh = 64 - (di * W + dj)
                    offs = _fold_offs(sh)
                    for piece in range(2):
                        cnt += 1
                        src_t = t + offs[piece]
                        if 0 <= src_t <= 16:
                            rhs = h_tiles[src_t][:, d * 64:(d + 1) * 64]
                        else:
                            rhs = t_zer[:, d * 64:(d + 1) * 64]
                        blk = ((set_idx * 9 + d) * 2 + piece) * 128
                        nc.tensor.matmul(py[:], t_fmx[:, blk:blk + 128], rhs,
                                         start=first, stop=(cnt == 18))
                        first = False
                nc.vector.tensor_copy(t_ysb[:, t * 64:(t + 1) * 64], py[:])

            u0 = 0
            for ci, Nc in enumerate(CHUNKS):
                ng = Nc // 128
                gA = [psA.tile([128, 320], F32, tag="gA", name=f"gA{ci}_{_g}") for _g in range(ng)]
                gB = [psB.tile([128, 260], F32, tag="gB", name=f"gB{ci}_{_g}") for _g in range(ng)]
                for lt in range(NT):
                    ps = ps3.tile([128, 384], F32, tag="s1ps")
                    for kc in range(5):
                        kb = KCH[kc]
                        nc.tensor.matmul(
                            ps[:, 0:Nc],
                            t_wn[0:kb, kc * L + lt * 128:kc * L + (lt + 1) * 128],
                            t_fpu[0:kb, kc * QH + u0:kc * QH + u0 + Nc],
                            start=(kc == 0), stop=(kc == 4))
                    et = ep.tile([128, 384], F32R, tag="et")
                    nc.scalar.activation(et[:, 0:Nc], ps[:, 0:Nc], AF.Exp,
                                         bias=t_bias[:], scale=SCALE)
                    wt_s = ws.tile([128, KD + 4], F32R, tag="wstream")
                    nc.sync.dma_start(wt_s[:], d_wT[lt])
                    for g in range(ng):
                        el = et[:, g * 128:(g + 1) * 128]
                        nc.tensor.matmul(gA[g][:], el, wt_s[:, 0:320],
                                         start=(lt == 0), stop=(lt == NT - 1))
                        nc.tensor.matmul(gB[g][:], el, wt_s[:, 320:KD + 4],
                                         start=(lt == 0), stop=(lt == NT - 1))
                for g in range(ng):
                    r = rp.tile([128, 1], F32, tag="rrec")
                    nc.vector.reciprocal(r[:], gB[g][:, 256:257])
                    ht = hp.tile([128, KD], F32R, tag="H")
                    nc.vector.tensor_scalar_mul(ht[:, 0:320], gA[g][:], r[:])
                    nc.vector.tensor_scalar_mul(ht[:, 320:KD], gB[g][:, 0:256], r[:])
                    h_tiles.append(ht)
                    # fold all y-tiles whose window is now complete
                    while fold_state["n"] <= len(h_tiles) - 3 and fold_state["n"] < 16:
                        fold(fold_state["n"])
                        fold_state["n"] += 1
                u0 += Nc
            while fold_state["n"] < 16:
                fold(fold_state["n"])
                fold_state["n"] += 1
            nc.sync.dma_start(d_y, t_ysb[:])

    nc.compile()
    return nc


# ---------------------------------------------------------------- host side
def _pool_mask(m):
    return m.reshape(H, 4, W, 4).mean(axis=(1, 3)).astype(np.float32)


def _build_foldm(h):
    """[54, 128, 128] fold matrices for half h (0: top, 1: bottom)."""
    out = np.zeros((3, 9, 2, 128, 128), np.float32)
    for d, (di, dj) in enumerate(DELTAS):
        sh = 64 - (di * W + dj)
        # validity per output row m
        v = np.ones(128, np.float32)
        px = np.arange(128) % 64
        if dj == 1:
            v[px == 0] = 0.0
        elif dj == -1:
            v[px == 63] = 0.0
        for set_idx in range(3):
            ve = v.copy()
            if set_idx == 0 and h == 0 and di == 1:
                ve[0:64] = 0.0
            if set_idx == 2 and h == 1 and di == -1:
                ve[64:128] = 0.0
            M0 = np.zeros((128, 128), np.float32)
            M1 = np.zeros((128, 128), np.float32)
            if sh == -1:
                M0[127, 0] = ve[0]
                for m in range(1, 128):
                    M1[m - 1, m] = ve[m]
            elif 0 <= sh <= 127:
                for m in range(0, 128 - sh):
                    M0[m + sh, m] = ve[m]
                for m in range(128 - sh, 128):
                    M1[m + sh - 128, m] = ve[m]
            elif sh == 128:
                for m in range(128):
                    M0[m, m] = ve[m]
            else:  # sh == 129
                for m in range(0, 127):
                    M0[m + 1, m] = ve[m]
                M1[0, 127] = ve[127]
            out[set_idx, d, 0] = M0
            out[set_idx, d, 1] = M1
    return out.reshape(54, 128, 128)


def _unfold_np(x, dtype=np.float64):
    """x: [C, H, W] -> [L, C*9] with reference (c, i, j) ordering."""
    Cc = x.shape[0]
    xp = np.zeros((Cc, H + 2, W + 2), dtype)
    xp[:, 1:-1, 1:-1] = x
    cols = np.zeros((L, Cc, 3, 3), dtype)
    for i in range(3):
        for j in range(3):
            cols[:, :, i, j] = xp[:, i:i + H, j:j + W].reshape(Cc, L).T
    return cols.reshape(L, Cc * 9)


def _rescue(b, f, mask, offsets, flagged):
    """fp64 full-row rescore of flagged (s, q) queries."""
    by_s = {}
    for s, q in flagged:
        by_s.setdefault(s, []).append(q)
    mp0 = _pool_mask(np.asarray(mask[0, 0], np.float64)).astype(np.float64)
    mp0_unf = _unfold_np(mp0[None], np.float64)  # [L, 9]
    for s, qs in by_s.items():
        mps = _pool_mask(np.asarray(mask[s, 0], np.float64)).astype(np.float64)
        b2 = np.asarray(b[s], np.float64).reshape(C, H, W)
        f2 = np.asarray(f[s], np.float64).reshape(C, H, W)
        bu = _unfold_np(b2)  # [L, 576], (c, i, j) ordering
        w = bu.reshape(L, C, 9) * mp0_unf[:, None, :]
        wn = w.reshape(L, C * 9) / np.sqrt((w ** 2).sum(axis=(1, 2)) + EPS)[:, None]
        fm = f2 * (1.0 - mps)[None]
        fu = _unfold_np(fm)  # [L, 576]
        qs_arr = np.array(qs)
        scores = wn @ fu[qs_arr].T  # [L, nq]
        am = scores.argmax(axis=0)
        for k, q in enumerate(qs):
            offsets[s, 0, q // W, q % W] = am[k]


def kernel(b, f, mask):
    global _BUILT
    b = np.ascontiguousarray(np.asarray(b, np.float32))
    f = np.ascontiguousarray(np.asarray(f, np.float32))
    mask = np.ascontiguousarray(np.asarray(mask, np.float32))
    if _BUILT is None:
        _BUILT = _build()
    nc = _BUILT

    ident = np.eye(128, dtype=np.float32)
    onesr = np.ones((128, 32), np.float32)
    foldm_h = {hh: _build_foldm(hh) for hh in (0, 1)}
    mp0 = _pool_mask(mask[0, 0])
    mp0pad = np.zeros((66, 66), np.float32)
    mp0pad[1:-1, 1:-1] = mp0

    in_maps = []
    for core in range(NCORES):
        s, hh = divmod(core, 2)
        q0 = hh * QOWN
        fsl = np.zeros((C, FSL), np.float32)
        omsl = np.zeros((1, FSL), np.float32)
        lo, hi = q0 - 192, q0 + 2240
        slo, shi = max(lo, 0), min(hi, L)
        f2 = f[s].reshape(C, L)
        fsl[:, slo - lo:shi - lo] = f2[:, slo:shi]
        mps = _pool_mask(mask[s, 0]).reshape(L)
        omsl[0, slo - lo:shi - lo] = 1.0 - mps[slo:shi]
        in_maps.append({
            "b_in": b[s].reshape(C, L),
            "f_sl": fsl,
            "onem": omsl,
            "mp0pad": mp0pad.reshape(-1),
            "identr": ident,
            "onesr": onesr,
            "foldm": foldm_h[hh],
        })

    import os
    trace = bool(int(os.environ.get("KERNEL_TRACE", "0")))
    res = run_bass_kernel_spmd(nc, in_maps, list(range(NCORES)), trace=trace)
    global _LAST_RES
    _LAST_RES = res

    y = np.zeros((B, C, H, W), np.float32)
    offsets = np.zeros((B, 1, H, W), np.int32)
    flagged = []
    for core in range(NCORES):
        s, hh = divmod(core, 2)
        r = res.results[core]
        yt = r["y_out"].reshape(128, 16, 64).transpose(1, 0, 2).reshape(QOWN, 64)
        y[s, :, hh * 32:(hh + 1) * 32, :] = yt.T.reshape(C, 32, W)
        ix = r["idx_out"].reshape(128, 16, 8)
        mx = r["max_out"].reshape(128, 16, 8)
        top = ix[:, :, 0].astype(np.int64).T.reshape(QOWN)  # [q]
        offsets[s, 0, hh * 32:(hh + 1) * 32, :] = top.reshape(32, W)
        gap = (mx[:, :, 0] - mx[:, :, 1]).T.reshape(QOWN)
        for ql in np.nonzero(gap < GAP_RESCUE)[0]:
            q = hh * QOWN + int(ql)
            flagged.append((s, q))

    if flagged:
        _rescue(b, f, mask, offsets, flagged)

    return y, offsets


# revision 14
# speedup vs baseline: 1.2097x; 1.2097x over previous
"""ContextualAttention TRN2 kernel (8 NeuronCores, SPMD).

Sharding: core = (sample s, query-half h); s = core // 2, h = core % 2.
Each core handles its sample's scores/softmax/attention for queries in rows
[32h, 32h+32) of the 64x64 query grid (plus a +-64-query halo for the
conv-transpose fold) and the argmax ("offsets") for its own rows.

Device pipeline (all heavy matmuls f32r = 1 cyc/row on the PE):
  P0  fm = f_slice * (1 - mp_s) (broadcast via GPSIMD), b rounded + PE-
      transposed, staged to DRAM as b^T padded.
  P1  patch bank: per l-tile, gather 9 shifted b^T slices, * m0 (stride-0
      broadcast) -> w^T [l, (d,c)] + ones col; norm^2 via ACT Square-accum;
      rn = 1/sqrt(n2+eps); wn^T = w^T * rn; PE-transpose -> wn [(d,c), l];
      w^T tile streamed to DRAM.  fp_unf built by 9 shifted DMAs from fm.
  P2  GEMM1a: score[q, l] per q-tile -> DVE top-8 max / argmax; global max
      -> softmax shift bias = 40 - 10*gmax.
  P3  per q-chunk x l-tile: GEMM1b score^T[l, q] -> ACT exp(10*s + bias) ->
      e^T (f32r); GEMM2 accumulates G[q, (d,c)|den] = e^T.T @ w^T; then
      H = G * (1/den).
  P4  fold: y^T[p, c] = sum_d H[p - dlin(d), (d,c)] via masked shifted-
      diagonal PE matmuls (host-built masks encode x-wrap + sample edges).

Host: shards inputs, pools the masks (65k flops), builds constants,
reassembles outputs, and re-resolves argmax for near-tie queries
(device top-2 gap < 3e-3) with an exact fp64 rescore.
"""
import numpy as np
from contextlib import ExitStack

import concourse.bass as bass
import concourse.bacc as bacc
import concourse.tile as tile
import concourse.mybir as mybir
import concourse.bass_isa as bass_isa
from concourse.bass_utils import run_bass_kernel_spmd

F32 = mybir.dt.float32
F32R = mybir.dt.float32r
U32 = mybir.dt.uint32
AF = mybir.ActivationFunctionType
ALU = mybir.AluOpType

B, C, H, W = 4, 64, 64, 64
L = H * W                       # 4096
NCORES = 8
KD = 9 * C                      # 576
KCH = [128, 128, 128, 128, 64]
NT = 32                         # l-tiles
QOWN = L // 2                   # 2048 own queries per core
QH = QOWN + 2 * W               # 2176 incl halo
NQT = QH // 128                 # 17
FSL = 2432                      # f slice width: q in [q0-192, q0+2240)
CHUNKS = [384, 384, 384, 384, 384, 256]
SCALE = 10.0
EPS = 1e-4
DELTAS = [(di, dj) for di in (-1, 0, 1) for dj in (-1, 0, 1)]
GAP_RESCUE = 3e-3

# fold piece table: per delta, shift sh = 64 - dlin and the two source-tile
# offsets (relative to the output y-tile index in H-local tiles)
def _fold_offs(sh):
    if sh == -1:
        return (-1, 0)
    if 0 <= sh <= 127:
        return (0, 1)
    return (1, 2)  # sh in {128, 129}

_BUILT = None
_LAST_RES = None


def _build():
    nc = bacc.Bacc("TRN2", target_bir_lowering=False, debug=False,
                   num_devices=NCORES)

    d_b = nc.dram_tensor("b_in", [C, L], F32, kind="ExternalInput").ap()
    d_f = nc.dram_tensor("f_sl", [C, FSL], F32, kind="ExternalInput").ap()
    d_om = nc.dram_tensor("onem", [1, FSL], F32, kind="ExternalInput").ap()
    d_mp0 = nc.dram_tensor("mp0pad", [66 * 66], F32, kind="ExternalInput").ap()
    d_id = nc.dram_tensor("identr", [128, 128], F32R, kind="ExternalInput").ap()
    d_1s = nc.dram_tensor("onesr", [128, 32], F32R, kind="ExternalInput").ap()
    d_fmx = nc.dram_tensor("foldm", [54, 128, 128], F32R, kind="ExternalInput").ap()

    d_y = nc.dram_tensor("y_out", [128, 16 * 64], F32, kind="ExternalOutput").ap()
    d_ix = nc.dram_tensor("idx_out", [128, 16 * 8], U32, kind="ExternalOutput").ap()
    d_mx = nc.dram_tensor("max_out", [128, 16 * 8], F32, kind="ExternalOutput").ap()

    d_bpT = nc.dram_tensor("bpadT", [(L + 4 * W) * C], F32R, kind="Internal").ap()
    d_wT = nc.dram_tensor("wT_dram", [NT, 128, KD + 4], F32R, kind="Internal").ap()

    with tile.TileContext(nc) as tc, ExitStack() as ctx:
        cst = ctx.enter_context(tc.tile_pool(name="cst", bufs=1))
        t_id = cst.tile([128, 128], F32R, tag="ident")
        nc.sync.dma_start(t_id[:], d_id)
        t_1s = cst.tile([128, 32], F32R, tag="onesr")
        nc.sync.dma_start(t_1s[:], d_1s)
        t_zer = cst.tile([128, KD], F32R, tag="zerot")
        nc.vector.memset(t_zer[:].bitcast(U32), 0)

        bigp = ctx.enter_context(tc.tile_pool(name="bigp", bufs=1))
        t_wn = bigp.tile([128, 5 * L], F32R, tag="wn")
        t_fpu = bigp.tile([128, 5 * QH], F32R, tag="fpu")

        # ---------------- P0: fm + b staging ----------------
        with tc.tile_pool(name="pre", bufs=1) as pre, \
             tc.tile_pool(name="pps", bufs=3, space="PSUM") as pps:
            t_fmp = pre.tile([64, FSL], F32R, tag="fmp")
            t_f = pre.tile([64, FSL], F32, tag="fload")
            nc.sync.dma_start(t_f[:], d_f)
            t_omr = pre.tile([1, FSL], F32, tag="omrow")
            nc.sync.dma_start(t_omr[:], d_om)
            t_omb = pre.tile([64, FSL], F32, tag="omrep")
            nc.gpsimd.partition_broadcast(t_omb[:], t_omr[:], channels=64)
            nc.vector.tensor_tensor(t_fmp[:], t_f[:], t_omb[:], ALU.mult)

            t_b = pre.tile([64, L], F32, tag="bload")
            nc.sync.dma_start(t_b[:], d_b)
            t_br = pre.tile([64, L], F32R, tag="bround")
            nc.vector.tensor_copy(t_br[:], t_b[:])
            t_bT = pre.tile([128, NT * 64], F32R, tag="bT")
            for t in range(NT):
                pt = pps.tile([128, 64], F32R, tag="ptr0")
                nc.tensor.transpose(pt[:], t_br[:, t * 128:(t + 1) * 128], t_id[0:64, 0:64])
                if t % 2 == 0:
                    nc.vector.tensor_copy(t_bT[:, t * 64:(t + 1) * 64], pt[:])
                else:
                    nc.scalar.copy(t_bT[:, t * 64:(t + 1) * 64], pt[:])
            # bpadT[64 + l, c] with zero pad rows
            nc.sync.dma_start(
                bass.AP(tensor=d_bpT.tensor, offset=0, ap=[[64, 128], [1, 64]]),
                t_zer[:, 0:64])
            nc.sync.dma_start(
                bass.AP(tensor=d_bpT.tensor, offset=(2 * W + L) * C,
                        ap=[[64, 128], [1, 64]]),
                t_zer[:, 0:64])
            nc.sync.dma_start(
                bass.AP(tensor=d_bpT.tensor, offset=2 * W * C,
                        ap=[[C, 128], [128 * C, NT], [1, 64]]),
                t_bT[:].rearrange("p (t c) -> p t c", t=NT))

            # fp_unf: 9 shifted DMAs from fm (+ x-wrap zeroing)
            for d, (di, dj) in enumerate(DELTAS):
                dlin = di * W + dj
                kc, half = divmod(d, 2)
                r0 = half * 64
                nc.sync.dma_start(
                    t_fpu[r0:r0 + 64, kc * QH:kc * QH + QH],
                    t_fmp[:, 128 + dlin:128 + dlin + QH])
            for d, (di, dj) in enumerate(DELTAS):
                if dj == 0:
                    continue
                kc, half = divmod(d, 2)
                r0 = half * 64
                col0 = 63 if dj == 1 else 0
                nc.vector.memset(
                    t_fpu[r0:r0 + 64, kc * QH + col0:kc * QH + QH:64].bitcast(U32), 0)

        # ---------------- P1: patch bank ----------------
        with tc.tile_pool(name="p1", bufs=1) as p1, \
             tc.tile_pool(name="btg", bufs=3) as btg, \
             tc.tile_pool(name="wtt", bufs=3) as wtt, \
             tc.tile_pool(name="wps", bufs=4, space="PSUM") as wps:
            t_m0T = p1.tile([128, 9 * NT], F32, tag="m0T")
            for d, (di, dj) in enumerate(DELTAS):
                off0 = (1 + di) * 66 + (1 + dj)
                for ph in range(2):
                    g = bass.AP(tensor=d_mp0.tensor, offset=off0 + ph * 66,
                                ap=[[1, 64], [132, 32]])
                    nc.sync.dma_start(
                        t_m0T[ph * 64:(ph + 1) * 64, d * 32:(d + 1) * 32], g)

            t_n2 = p1.tile([128, NT], F32, tag="n2")
            t_rn = p1.tile([128, NT], F32, tag="rn")
            t_eps = p1.tile([128, 1], F32, tag="eps")
            nc.vector.memset(t_eps[:], EPS)
            GRP = 8
            bt8s = {}
            for rr in range(NT // GRP):
                bt8 = btg.tile([128, GRP * KD], F32R, tag="btg", name=f"bt8_{rr}")
                for d, (di, dj) in enumerate(DELTAS):
                    dlin = di * W + dj
                    nc.sync.dma_start(
                        bt8[:].rearrange("p (g k) -> p g k", g=GRP)[:, :, d * 64:(d + 1) * 64],
                        bass.AP(tensor=d_bpT.tensor,
                                offset=(2 * W + dlin + 128 * (GRP * rr)) * C,
                                ap=[[C, 128], [128 * C, GRP], [1, 64]]))
                bt8s[rr] = bt8
            for t in range(NT):
                bt = bt8s[t // GRP][:, (t % GRP) * KD:(t % GRP + 1) * KD]
                wt = wtt.tile([128, KD + 4], F32R, tag="wTt")
                nc.vector.memset(wt[:, KD + 1:KD + 4].bitcast(U32), 0)
                nc.vector.tensor_tensor(
                    wt[:, 0:KD].rearrange("p (d c) -> p d c", d=9),
                    bt.rearrange("p (d c) -> p d c", d=9),
                    t_m0T[:, t:9 * NT:NT].rearrange("p (d u) -> p d u", u=1)
                        .broadcast_to([128, 9, 64]),
                    ALU.mult)
                nc.vector.tensor_copy(wt[:, KD:KD + 1], t_1s[:, 0:1])
                sq = wtt.tile([128, KD], F32, tag="sqscr")
                nc.scalar.activation(sq[:], wt[:, 0:KD], AF.Square,
                                     bias=0.0, scale=1.0,
                                     accum_out=t_n2[:, t:t + 1])
                nc.scalar.activation(t_rn[:, t:t + 1], t_n2[:, t:t + 1], AF.Sqrt,
                                     bias=t_eps[:], scale=1.0)
                nc.vector.reciprocal(t_rn[:, t:t + 1], t_rn[:, t:t + 1])
                wnt = wtt.tile([128, KD], F32R, tag="wnTt")
                nc.vector.tensor_scalar_mul(wnt[:], wt[:, 0:KD], t_rn[:, t:t + 1])
                for kc in range(5):
                    kb = KCH[kc]
                    ptr = wps.tile([128, 128], F32R, tag="ptrw")
                    nc.tensor.transpose(ptr[0:kb, :],
                                        wnt[:, kc * 128:kc * 128 + kb], t_id[:])
                    dst = t_wn[0:kb, kc * L + t * 128:kc * L + (t + 1) * 128]
                    if t % 2 == 0:
                        nc.vector.tensor_copy(dst, ptr[0:kb, :])
                    else:
                        nc.scalar.copy(dst, ptr[0:kb, :])
                nc.sync.dma_start(d_wT[t], wt[:])

        # ---------------- P2: score[q, l] + argmax ----------------
        t_bias = cst.tile([128, 1], F32, tag="bias")
        with tc.tile_pool(name="p2", bufs=2) as p2, \
             tc.tile_pool(name="p2s", bufs=1) as p2s, \
             tc.tile_pool(name="ps2", bufs=8, space="PSUM") as ps2:
            t_mxa = p2s.tile([128, 16 * 8], F32, tag="mxall")
            t_ixa = p2s.tile([128, 16 * 8], U32, tag="ixall")
            for qt in range(16):
                sc = p2.tile([128, L], F32, tag="scq")
                for lc in range(8):
                    ps = ps2.tile([128, 512], F32, tag="scps")
                    for kc in range(5):
                        kb = KCH[kc]
                        nc.tensor.matmul(
                            ps[:],
                            t_fpu[0:kb, kc * QH + 64 + qt * 128:
                                  kc * QH + 64 + (qt + 1) * 128],
                            t_wn[0:kb, kc * L + lc * 512:kc * L + (lc + 1) * 512],
                            start=(kc == 0), stop=(kc == 4))
                    nc.scalar.copy(sc[:, lc * 512:(lc + 1) * 512], ps[:])
                nc.vector.max(t_mxa[:, qt * 8:(qt + 1) * 8], sc[:])
                nc.vector.max_index(t_ixa[:, qt * 8:(qt + 1) * 8],
                                    t_mxa[:, qt * 8:(qt + 1) * 8], sc[:])
            nc.sync.dma_start(d_ix, t_ixa[:])
            nc.sync.dma_start(d_mx, t_mxa[:])
            # global max -> exp bias = 40 - 10*gmax
            t_g8 = p2s.tile([128, 8], F32, tag="g8")
            nc.vector.max(t_g8[:], t_mxa[:])
            t_gr = p2s.tile([128, 1], F32, tag="gmaxr")
            nc.gpsimd.partition_all_reduce(t_gr[:], t_g8[:, 0:1], channels=128,
                                           reduce_op=bass_isa.ReduceOp.max)
            nc.vector.tensor_scalar(t_bias[:], t_gr[:], -SCALE, 40.0,
                                    ALU.mult, ALU.add)

        # ---------------- P3: exp + GEMM2 + H; P4: fold ----------------
        fold_state = {"n": 0}
        h_tiles = []
        t_ysb = cst.tile([128, 16 * 64], F32, tag="ysb")

        with tc.tile_pool(name="fmx", bufs=1) as fmxp, \
             tc.tile_pool(name="ep", bufs=3) as ep, \
             tc.tile_pool(name="ws", bufs=3) as ws, \
             tc.tile_pool(name="hp", bufs=6) as hp, \
             tc.tile_pool(name="rp", bufs=4) as rp, \
             tc.tile_pool(name="ps3", bufs=2, space="PSUM") as ps3, \
             tc.tile_pool(name="psA", bufs=3, space="PSUM") as psA, \
             tc.tile_pool(name="psB", bufs=3, space="PSUM") as psB:
            t_fmx = fmxp.tile([128, 54 * 128], F32R, tag="foldm")
            nc.sync.dma_start(
                t_fmx[:].rearrange("p (m k) -> p m k", m=54),
                d_fmx.rearrange("m p k -> p m k"))

            def fold(t):
                # shares the "s1ps" slots with GEMM1b psum (8-bank budget)
                py = ps3.tile([128, 64], F32, tag="s1ps")
                first = True
                set_idx = 0 if t == 0 else (2 if t == 15 else 1)
                cnt = 0
                pieces_all = []
                for d, (di, dj) in enumerate(DELTAS):
                    sh = 64 - (di * W + dj)
                    offs = _fold_offs(sh)
                    for piece in range(2):
                        if sh in (0, 128) and piece == 1:
                            continue  # structurally zero matrix
                        pieces_all.append((d, offs, piece))
                n_mm = len(pieces_all)
                for d, offs, piece in pieces_all:
                    if True:
                        cnt += 1
                        src_t = t + offs[piece]
                        if 0 <= src_t <= 16:
                            rhs = h_tiles[src_t][:, d * 64:(d + 1) * 64]
                        else:
                            rhs = t_zer[:, d * 64:(d + 1) * 64]
                        blk = ((set_idx * 9 + d) * 2 + piece) * 128
                        nc.tensor.matmul(py[:], t_fmx[:, blk:blk + 128], rhs,
                                         start=first, stop=(cnt == n_mm))
                        first = False
                nc.vector.tensor_copy(t_ysb[:, t * 64:(t + 1) * 64], py[:])

            u0 = 0
            for ci, Nc in enumerate(CHUNKS):
                ng = Nc // 128
                gA = [psA.tile([128, 320], F32, tag="gA", name=f"gA{ci}_{_g}") for _g in range(ng)]
                gB = [psB.tile([128, 260], F32, tag="gB", name=f"gB{ci}_{_g}") for _g in range(ng)]
                for lt in range(NT):
                    ps = ps3.tile([128, 384], F32, tag="s1ps")
                    for kc in range(5):
                        kb = KCH[kc]
                        nc.tensor.matmul(
                            ps[:, 0:Nc],
                            t_wn[0:kb, kc * L + lt * 128:kc * L + (lt + 1) * 128],
                            t_fpu[0:kb, kc * QH + u0:kc * QH + u0 + Nc],
                            start=(kc == 0), stop=(kc == 4))
                    et = ep.tile([128, 384], F32R, tag="et")
                    nc.scalar.activation(et[:, 0:Nc], ps[:, 0:Nc], AF.Exp,
                                         bias=t_bias[:], scale=SCALE)
                    if lt % 2 == 0:
                        wt_s2 = ws.tile([128, 2 * (KD + 4)], F32R, tag="wstream",
                                        name=f"ws{ci}_{lt}")
                        nc.sync.dma_start(
                            wt_s2[:].rearrange("p (g k) -> p g k", g=2),
                            d_wT[lt:lt + 2].rearrange("g p k -> p g k"))
                    wo = (lt % 2) * (KD + 4)
                    wt_s = wt_s2[:, wo:wo + KD + 4]
                    for g in range(ng):
                        el = et[:, g * 128:(g + 1) * 128]
                        nc.tensor.matmul(gA[g][:], el, wt_s[:, 0:320],
                                         start=(lt == 0), stop=(lt == NT - 1))
                        nc.tensor.matmul(gB[g][:], el, wt_s[:, 320:KD + 4],
                                         start=(lt == 0), stop=(lt == NT - 1))
                for g in range(ng):
                    r = rp.tile([128, 1], F32, tag="rrec")
                    nc.vector.reciprocal(r[:], gB[g][:, 256:257])
                    ht = hp.tile([128, KD], F32R, tag="H")
                    nc.vector.tensor_scalar_mul(ht[:, 0:320], gA[g][:], r[:])
                    nc.vector.tensor_scalar_mul(ht[:, 320:KD], gB[g][:, 0:256], r[:])
                    h_tiles.append(ht)
                    # fold all y-tiles whose window is now complete
                    while fold_state["n"] <= len(h_tiles) - 3 and fold_state["n"] < 16:
                        fold(fold_state["n"])
                        fold_state["n"] += 1
                u0 += Nc
            while fold_state["n"] < 16:
                fold(fold_state["n"])
                fold_state["n"] += 1
            nc.sync.dma_start(d_y, t_ysb[:])

    nc.compile()
    return nc


# ---------------------------------------------------------------- host side
def _pool_mask(m):
    return m.reshape(H, 4, W, 4).mean(axis=(1, 3)).astype(np.float32)


def _build_foldm(h):
    """[54, 128, 128] fold matrices for half h (0: top, 1: bottom)."""
    out = np.zeros((3, 9, 2, 128, 128), np.float32)
    for d, (di, dj) in enumerate(DELTAS):
        sh = 64 - (di * W + dj)
        # validity per output row m
        v = np.ones(128, np.float32)
        px = np.arange(128) % 64
        if dj == 1:
            v[px == 0] = 0.0
        elif dj == -1:
            v[px == 63] = 0.0
        for set_idx in range(3):
            ve = v.copy()
            if set_idx == 0 and h == 0 and di == 1:
                ve[0:64] = 0.0
            if set_idx == 2 and h == 1 and di == -1:
                ve[64:128] = 0.0
            M0 = np.zeros((128, 128), np.float32)
            M1 = np.zeros((128, 128), np.float32)
            if sh == -1:
                M0[127, 0] = ve[0]
                for m in range(1, 128):
                    M1[m - 1, m] = ve[m]
            elif 0 <= sh <= 127:
                for m in range(0, 128 - sh):
                    M0[m + sh, m] = ve[m]
                for m in range(128 - sh, 128):
                    M1[m + sh - 128, m] = ve[m]
            elif sh == 128:
                for m in range(128):
                    M0[m, m] = ve[m]
            else:  # sh == 129
                for m in range(0, 127):
                    M0[m + 1, m] = ve[m]
                M1[0, 127] = ve[127]
            out[set_idx, d, 0] = M0
            out[set_idx, d, 1] = M1
    return out.reshape(54, 128, 128)


def _unfold_np(x, dtype=np.float64):
    """x: [C, H, W] -> [L, C*9] with reference (c, i, j) ordering."""
    Cc = x.shape[0]
    xp = np.zeros((Cc, H + 2, W + 2), dtype)
    xp[:, 1:-1, 1:-1] = x
    cols = np.zeros((L, Cc, 3, 3), dtype)
    for i in range(3):
        for j in range(3):
            cols[:, :, i, j] = xp[:, i:i + H, j:j + W].reshape(Cc, L).T
    return cols.reshape(L, Cc * 9)


def _rescue(b, f, mask, offsets, flagged):
    """fp64 full-row rescore of flagged (s, q) queries."""
    by_s = {}
    for s, q in flagged:
        by_s.setdefault(s, []).append(q)
    mp0 = _pool_mask(np.asarray(mask[0, 0], np.float64)).astype(np.float64)
    mp0_unf = _unfold_np(mp0[None], np.float64)  # [L, 9]
    for s, qs in by_s.items():
        mps = _pool_mask(np.asarray(mask[s, 0], np.float64)).astype(np.float64)
        b2 = np.asarray(b[s], np.float64).reshape(C, H, W)
        f2 = np.asarray(f[s], np.float64).reshape(C, H, W)
        bu = _unfold_np(b2)  # [L, 576], (c, i, j) ordering
        w = bu.reshape(L, C, 9) * mp0_unf[:, None, :]
        wn = w.reshape(L, C * 9) / np.sqrt((w ** 2).sum(axis=(1, 2)) + EPS)[:, None]
        fm = f2 * (1.0 - mps)[None]
        fu = _unfold_np(fm)  # [L, 576]
        qs_arr = np.array(qs)
        scores = wn @ fu[qs_arr].T  # [L, nq]
        am = scores.argmax(axis=0)
        for k, q in enumerate(qs):
            offsets[s, 0, q // W, q % W] = am[k]


def kernel(b, f, mask):
    global _BUILT
    b = np.ascontiguousarray(np.asarray(b, np.float32))
    f = np.ascontiguousarray(np.asarray(f, np.float32))
    mask = np.ascontiguousarray(np.asarray(mask, np.float32))
    if _BUILT is None:
        _BUILT = _build()
    nc = _BUILT

    ident = np.eye(128, dtype=np.float32)
    onesr = np.ones((128, 32), np.float32)
    foldm_h = {hh: _build_foldm(hh) for hh in (0, 1)}
    mp0 = _pool_mask(mask[0, 0])
    mp0pad = np.zeros((66, 66), np.float32)
    mp0pad[1:-1, 1:-1] = mp0

    in_maps = []
    for core in range(NCORES):
        s, hh = divmod(core, 2)
        q0 = hh * QOWN
        fsl = np.zeros((C, FSL), np.float32)
        omsl = np.zeros((1, FSL), np.float32)
        lo, hi = q0 - 192, q0 + 2240
        slo, shi = max(lo, 0), min(hi, L)
        f2 = f[s].reshape(C, L)
        fsl[:, slo - lo:shi - lo] = f2[:, slo:shi]
        mps = _pool_mask(mask[s, 0]).reshape(L)
        omsl[0, slo - lo:shi - lo] = 1.0 - mps[slo:shi]
        in_maps.append({
            "b_in": b[s].reshape(C, L),
            "f_sl": fsl,
            "onem": omsl,
            "mp0pad": mp0pad.reshape(-1),
            "identr": ident,
            "onesr": onesr,
            "foldm": foldm_h[hh],
        })

    import os
    trace = bool(int(os.environ.get("KERNEL_TRACE", "0")))
    res = run_bass_kernel_spmd(nc, in_maps, list(range(NCORES)), trace=trace)
    global _LAST_RES
    _LAST_RES = res

    y = np.zeros((B, C, H, W), np.float32)
    offsets = np.zeros((B, 1, H, W), np.int32)
    flagged = []
    for core in range(NCORES):
        s, hh = divmod(core, 2)
        r = res.results[core]
        yt = r["y_out"].reshape(128, 16, 64).transpose(1, 0, 2).reshape(QOWN, 64)
        y[s, :, hh * 32:(hh + 1) * 32, :] = yt.T.reshape(C, 32, W)
        ix = r["idx_out"].reshape(128, 16, 8)
        mx = r["max_out"].reshape(128, 16, 8)
        top = ix[:, :, 0].astype(np.int64).T.reshape(QOWN)  # [q]
        offsets[s, 0, hh * 32:(hh + 1) * 32, :] = top.reshape(32, W)
        gap = (mx[:, :, 0] - mx[:, :, 1]).T.reshape(QOWN)
        for ql in np.nonzero(gap < GAP_RESCUE)[0]:
            q = hh * QOWN + int(ql)
            flagged.append((s, q))

    if flagged:
        _rescue(b, f, mask, offsets, flagged)

    return y, offsets
